# revision 19
# baseline (speedup 1.0000x reference)
"""Trainium2 Bass kernel v2 for nn_AutoregressiveDecoder.

B=64, L=128, E=512, H=512, V=32000, T=64, 8 NeuronCores.

Design (vs v1):
- fp32 logits (exact) -> local argmax needs NO top-3 re-evaluation chain.
- emb is SHARDED (8MB/core); each core gathers the embedding row of its own
  local winner and ships it inside the per-step exchange payload, so no
  replicated emb (64MB) or mtab (196MB) input is needed.
- Exchange payload per core: [64, 514] f32 = { x_row[0:512], exact_max, gid },
  AllGathered across the 8 cores each step.  Global winner = max over the 8
  slot values, ties broken by min gid (matches the reference argmax
  first-index tiebreak since gid = rank*4000 + local_idx); the winner's
  embedding row is selected from the payload slots by a one-hot mask, so no
  replicated emb table is needed.
- Output logits are converted to bf16 on-device, upcast to f32 on host.
- GRU weights ship sharded and are AllGathered on-device at startup.
- Input tensors are few and small: ~17.5MB/core total.
"""
import sys

if "/opt/trn_rl_repo" not in sys.path:
    sys.path.insert(0, "/opt/trn_rl_repo")

import numpy as np

import concourse.bass as bass
import concourse.mybir as mybir
import concourse.bacc as bacc
import concourse.tile as tile
import concourse.bass_utils as bass_utils
from concourse.masks import make_identity

F32 = mybir.dt.float32
BF16 = mybir.dt.bfloat16
U32 = mybir.dt.uint32
I32 = mybir.dt.int32
AF = mybir.ActivationFunctionType
OP = mybir.AluOpType
AX = mybir.AxisListType

B, L, E, H, V, T = 64, 128, 512, 512, 32000, 64
NC_N = 8
VS = V // NC_N          # 4000 vocab per core
VSP = 4096              # padded (8 tiles of 512)
KC = H // 128           # 4 contraction chunks
NVT = VSP // 512        # 8 vocab tiles per core
NEG = -1.0e30
PAYW = 514              # 512 x + val + gid


def build(t_steps=T, no_cc=False, out_T=None, out_bf16=True, probe=None):
    # probe="noex": skip payload/exchange/global-select; x:=h (timing only)
    # probe="nologit": skip logits/argmax/exchange entirely; x:=h (timing only)
    if out_T is None:
        out_T = t_steps
    nc = bacc.Bacc("TRN2", target_bir_lowering=False, debug=False,
                   num_devices=NC_N)

    # ---------------- DRAM I/O ----------------
    # GRU weights arrive SHARDED (rows 64c:64c+64 of wihT stacked on the same
    # rows of whhT, [128, 1536] per core) and are AllGathered on-device once,
    # saving ~5.5MB/core of host->device transfer.
    d_wsh = nc.dram_tensor("wihwhh_sh", [128, 3 * H], F32,
                           kind="ExternalInput").ap()
    d_wfcT = nc.dram_tensor("wfcT", [H, VSP], F32, kind="ExternalInput").ap()
    d_wprojT = nc.dram_tensor("wprojT", [L, H], F32, kind="ExternalInput").ap()
    d_zT = nc.dram_tensor("zT", [L, B], F32, kind="ExternalInput").ap()
    d_bias_gi = nc.dram_tensor("bias_gi", [1, 3 * H], F32, kind="ExternalInput").ap()
    d_bias_hn = nc.dram_tensor("bias_hn", [1, H], F32, kind="ExternalInput").ap()
    d_bias_fc = nc.dram_tensor("bias_fc", [1, VSP], F32, kind="ExternalInput").ap()
    d_bias_proj = nc.dram_tensor("bias_proj", [1, H], F32, kind="ExternalInput").ap()
    d_rank = nc.dram_tensor("rank_col", [B, 1], F32, kind="ExternalInput").ap()
    d_embs = nc.dram_tensor("embs", [VS, E], F32, kind="ExternalInput").ap()
    out_dt = BF16 if out_bf16 else F32
    d_out = nc.dram_tensor("out", [B, out_T * VS], out_dt,
                           kind="ExternalOutput").ap()

    with tile.TileContext(nc) as tc:
        with tc.tile_pool(name="wts", bufs=1) as wpool, \
             tc.tile_pool(name="sb", bufs=2) as sb, \
             tc.tile_pool(name="sb1", bufs=1) as sb1, \
             tc.tile_pool(name="lgps", bufs=2, space="PSUM") as lgp, \
             tc.tile_pool(name="grups", bufs=1, space="PSUM") as grup, \
             tc.tile_pool(name="tps", bufs=2, space="PSUM") as tps, \
             tc.tile_pool(name="dr", bufs=2, space="DRAM") as dr:
            # ---------------- load weights ----------------
            wih = wpool.tile([128, KC * 3 * H], F32)
            whh = wpool.tile([128, KC * 3 * H], F32)
            wfc = wpool.tile([128, KC * VSP], F32)
            wproj = wpool.tile([128, H], F32)
            zT_sb = wpool.tile([128, B], F32)
            # on-device allgather of the sharded GRU weights (collectives
            # cannot read IO tensors directly -> bounce via Internal DRAM)
            wsh_int = dr.tile([128, 3 * H], F32, tag="wshint")
            nc.sync.dma_start(wsh_int[:], d_wsh)
            wall = dr.tile([NC_N * 128, 3 * H], F32, tag="wall")
            nc.gpsimd.collective_compute(
                "AllGather", OP.bypass,
                replica_groups=[list(range(NC_N))],
                ins=[wsh_int[:].opt()], outs=[wall[:].opt()])
            # wall block c: rows [0:64] = wihT[64c:64c+64], [64:128] = whhT[...]
            for k in range(KC):
                ksl_g = slice(k * 3 * H, (k + 1) * 3 * H)
                nc.sync.dma_start(wih[0:64, ksl_g],
                                  wall[2 * k * 128:2 * k * 128 + 64, :])
                nc.sync.dma_start(wih[64:128, ksl_g],
                                  wall[(2 * k + 1) * 128:
                                       (2 * k + 1) * 128 + 64, :])
                nc.sync.dma_start(whh[0:64, ksl_g],
                                  wall[2 * k * 128 + 64:2 * k * 128 + 128, :])
                nc.sync.dma_start(whh[64:128, ksl_g],
                                  wall[(2 * k + 1) * 128 + 64:
                                       (2 * k + 1) * 128 + 128, :])
                nc.sync.dma_start(wfc[:, k * VSP:(k + 1) * VSP],
                                  d_wfcT[k * 128:(k + 1) * 128, :])
            nc.sync.dma_start(wproj[:], d_wprojT)
            nc.sync.dma_start(zT_sb[:], d_zT)
            b_gi = wpool.tile([1, 3 * H], F32)
            b_hn = wpool.tile([1, H], F32)
            b_fc = wpool.tile([1, VSP], F32)
            b_proj = wpool.tile([1, H], F32)
            rank_col = wpool.tile([B, 1], F32)
            nc.sync.dma_start(b_gi[:], d_bias_gi)
            nc.sync.dma_start(b_hn[:], d_bias_hn)
            nc.sync.dma_start(b_fc[:], d_bias_fc)
            nc.sync.dma_start(b_proj[:], d_bias_proj)
            nc.sync.dma_start(rank_col[:], d_rank)
            ident = wpool.tile([B, B], F32)
            make_identity(nc, ident[:])
            ones1 = wpool.tile([1, 128], F32)
            nc.vector.memset(ones1[:], 1.0)

            # ---------------- h0 ----------------
            h0_ps = lgp.tile([B, H], F32, tag="lg")
            nc.tensor.matmul(h0_ps[:], zT_sb[:], wproj[:], start=True, stop=False)
            nc.tensor.matmul(h0_ps[:], ones1[0:1, 0:B], b_proj[:],
                             start=False, stop=True)
            h_cur = sb.tile([B, H], F32, tag="h")
            nc.scalar.copy(h_cur[:], h0_ps[:])

            def transpose_to(dst_sb, src_top, src_bot=None):
                """4 PE transposes of a [64, 512] tensor (possibly split into
                two [64,256] partition-halves) into dst [128, KC*64]."""
                tp = tps.tile([128, 256], F32, tag="tp")
                for k in range(KC):
                    if src_bot is None:
                        src = src_top[:, k * 128:(k + 1) * 128]
                    else:
                        src = (src_top[:, (k % 2) * 128:(k % 2) * 128 + 128]
                               if k < 2 else
                               src_bot[:, (k % 2) * 128:(k % 2) * 128 + 128])
                    nc.tensor.transpose(tp[:, k * 64:(k + 1) * 64], src,
                                        ident[:])
                nc.scalar.copy(dst_sb[:], tp[:])

            hT = sb.tile([128, KC * 64], F32, tag="hT")
            transpose_to(hT, h_cur[:])
            xT = hT            # step 0: x = h0

            for t in range(t_steps):
                # ---------- GRU matmuls ----------
                rz_ps = grup.tile([B, 1024], F32, tag="rz")
                ghn_ps = grup.tile([B, 512], F32, tag="ghn")
                gin_ps = grup.tile([B, 512], F32, tag="gin")
                for j in range(2):
                    o = rz_ps[:, j * 512:(j + 1) * 512]
                    for k in range(KC):
                        nc.tensor.matmul(o, hT[:, k * 64:(k + 1) * 64],
                                         whh[:, k * 3 * H + j * 512:
                                             k * 3 * H + (j + 1) * 512],
                                         start=(k == 0), stop=False)
                    for k in range(KC):
                        nc.tensor.matmul(o, xT[:, k * 64:(k + 1) * 64],
                                         wih[:, k * 3 * H + j * 512:
                                             k * 3 * H + (j + 1) * 512],
                                         start=False, stop=False)
                    nc.tensor.matmul(o, ones1[0:1, 0:B],
                                     b_gi[:, j * 512:(j + 1) * 512],
                                     start=False, stop=True)
                for k in range(KC):
                    nc.tensor.matmul(ghn_ps[:], hT[:, k * 64:(k + 1) * 64],
                                     whh[:, k * 3 * H + 1024:k * 3 * H + 1536],
                                     start=(k == 0), stop=False)
                nc.tensor.matmul(ghn_ps[:], ones1[0:1, 0:B], b_hn[:],
                                 start=False, stop=True)
                for k in range(KC):
                    nc.tensor.matmul(gin_ps[:], xT[:, k * 64:(k + 1) * 64],
                                     wih[:, k * 3 * H + 1024:k * 3 * H + 1536],
                                     start=(k == 0), stop=False)
                nc.tensor.matmul(gin_ps[:], ones1[0:1, 0:B],
                                 b_gi[:, 1024:1536], start=False, stop=True)

                # ---------- gates ----------
                rz_sb = sb1.tile([B, 1024], F32, tag="rzsb")
                nc.scalar.activation(rz_sb[:], rz_ps[:], AF.Sigmoid)
                u_sb = sb1.tile([B, H], F32, tag="u")
                nc.vector.tensor_tensor(out=u_sb[:], in0=rz_sb[:, 0:512],
                                        in1=ghn_ps[:], op=OP.mult)
                nc.vector.tensor_tensor(out=u_sb[:], in0=u_sb[:],
                                        in1=gin_ps[:], op=OP.add)
                n_sb = sb1.tile([B, H], F32, tag="n")
                nc.scalar.activation(n_sb[:], u_sb[:], AF.Tanh)
                hmn = sb1.tile([B, H], F32, tag="hmn")
                nc.vector.tensor_tensor(out=hmn[:], in0=h_cur[:], in1=n_sb[:],
                                        op=OP.subtract)
                nc.vector.tensor_tensor(out=hmn[:], in0=rz_sb[:, 512:1024],
                                        in1=hmn[:], op=OP.mult)
                h_new = sb.tile([B, H], F32, tag="h")
                nc.vector.tensor_tensor(out=h_new[:], in0=n_sb[:], in1=hmn[:],
                                        op=OP.add)
                h_cur = h_new

                # ---------- hT ----------
                hT = sb.tile([128, KC * 64], F32, tag="hT")
                transpose_to(hT, h_cur[:])

                if probe == "nologit":
                    xT = hT
                    continue

                # ---------- logits (fp32, exact) ----------
                lg_sb = sb1.tile([B, VSP], F32, tag="lg_sb")
                tm8 = sb1.tile([B, NVT * 8], F32, tag="tm8")
                for v in range(NVT):
                    lg_ps = lgp.tile([B, 512], F32, tag="lg")
                    for k in range(KC):
                        nc.tensor.matmul(
                            lg_ps[:], hT[:, k * 64:(k + 1) * 64],
                            wfc[:, k * VSP + v * 512:k * VSP + (v + 1) * 512],
                            start=(k == 0), stop=False)
                    nc.tensor.matmul(lg_ps[:], ones1[0:1, 0:B],
                                     b_fc[:, v * 512:(v + 1) * 512],
                                     start=False, stop=True)
                    nc.scalar.copy(lg_sb[:, v * 512:(v + 1) * 512], lg_ps[:])
                    nc.vector.max(out=tm8[:, v * 8:(v + 1) * 8],
                                  in_=lg_sb[:, v * 512:(v + 1) * 512])

                # output DMA (bf16 on-device, host upcasts)
                to = t % out_T
                if out_bf16:
                    lg_bf = sb1.tile([B, VSP], BF16, tag="lg_bf")
                    nc.vector.tensor_copy(lg_bf[:], lg_sb[:])
                    nc.sync.dma_start(d_out[:, to * VS:(to + 1) * VS],
                                      lg_bf[:, 0:VS])
                else:
                    nc.sync.dma_start(d_out[:, to * VS:(to + 1) * VS],
                                      lg_sb[:, 0:VS])

                if t == t_steps - 1:
                    break       # no feedback needed after last step

                # ---------- local argmax (exact) ----------
                gmax = sb1.tile([B, 1], F32, tag="gmax")
                nc.vector.tensor_reduce(out=gmax[:], in_=tm8[:], axis=AX.X,
                                        op=OP.max)
                gmax8 = sb1.tile([B, 8], F32, tag="gmax8")
                nc.vector.tensor_copy(gmax8[:], gmax[:].to_broadcast([B, 8]))
                mi8 = sb1.tile([B, 8], U32, tag="mi8")
                nc.vector.max_index(out=mi8[:], in_max=gmax8[:],
                                    in_values=lg_sb[:])
                ids_l = sb1.tile([B, 1], I32, tag="ids_l")
                nc.vector.tensor_copy(ids_l[:], mi8[:, 0:1])

                if probe == "noex":
                    xT = hT
                    continue

                # ---------- payload: x + val + gid ----------
                pay = sb1.tile([B, PAYW], F32, tag="pay")
                nc.gpsimd.indirect_dma_start(
                    out=pay[:, 0:E], out_offset=None, in_=d_embs,
                    in_offset=bass.IndirectOffsetOnAxis(ap=ids_l[:, 0:1],
                                                        axis=0))
                nc.vector.tensor_copy(pay[:, E:E + 1], gmax[:])
                gidf = sb1.tile([B, 1], F32, tag="gidf")
                nc.vector.tensor_copy(gidf[:], mi8[:, 0:1])
                nc.vector.tensor_scalar(out=pay[:, E + 1:E + 2], in0=gidf[:],
                                        scalar1=rank_col[:, 0:1], scalar2=None,
                                        op0=OP.add)

                # ---------- exchange ----------
                cc_in = dr.tile([B, PAYW], F32, tag="ccin")
                cc_out = dr.tile([NC_N * B, PAYW], F32, tag="ccout")
                nc.gpsimd.dma_start(cc_in[:], pay[:])
                if no_cc:
                    for rr in range(NC_N):
                        nc.gpsimd.dma_start(
                            cc_out[rr * B:(rr + 1) * B, :], cc_in[:])
                else:
                    nc.gpsimd.collective_compute(
                        "AllGather", OP.bypass,
                        replica_groups=[list(range(NC_N))],
                        ins=[cc_in[:].opt()], outs=[cc_out[:].opt()])
                ag = sb1.tile([B, NC_N * PAYW], F32, tag="ag")
                nc.gpsimd.dma_start(
                    ag[:].rearrange("p (s w) -> p s w", s=NC_N),
                    cc_out[:].rearrange("(s p) w -> p s w", s=NC_N))

                # ---------- global select ----------
                ag3 = ag[:].rearrange("p (s w) -> p s w", s=NC_N)
                vals = ag3[:, :, E:E + 1].squeeze()        # [B, 8]
                gids = ag3[:, :, E + 1:E + 2].squeeze()    # [B, 8]
                gv = sb1.tile([B, 1], F32, tag="gv")
                nc.vector.tensor_reduce(out=gv[:], in_=vals, axis=AX.X,
                                        op=OP.max)
                mlt = sb1.tile([B, 8], F32, tag="mlt")
                nc.vector.tensor_scalar(out=mlt[:], in0=vals,
                                        scalar1=gv[:, 0:1], scalar2=None,
                                        op0=OP.is_lt)      # 1 where NOT max
                sel = sb1.tile([B, 8], F32, tag="sel")
                nc.vector.tensor_scalar(out=sel[:], in0=mlt[:],
                                        scalar1=2.0e9, scalar2=None,
                                        op0=OP.mult)
                nc.vector.tensor_tensor(out=sel[:], in0=sel[:], in1=gids,
                                        op=OP.add)
                smin = sb1.tile([B, 1], F32, tag="smin")
                nc.vector.tensor_reduce(out=smin[:], in_=sel[:], axis=AX.X,
                                        op=OP.min)
                # one-hot winner mask [B, 8]
                mone = sb1.tile([B, 8], F32, tag="mone")
                nc.vector.tensor_scalar(out=mone[:], in0=gids,
                                        scalar1=smin[:, 0:1], scalar2=None,
                                        op0=OP.is_equal)
                # xsel: multiply slots by mask, then sum over slots
                xs = ag3[:, :, 0:E]                        # [B, 8, 512]
                psel = sb1.tile([B, NC_N * E], F32, tag="lg_sb")
                ps3 = psel[:].rearrange("p (s w) -> p s w", s=NC_N)
                nc.vector.tensor_tensor(
                    out=ps3, in0=xs,
                    in1=mone[:].rearrange("p (s w) -> p s w",
                                          w=1).to_broadcast([B, NC_N, E]),
                    op=OP.mult)
                xsum = sb1.tile([B, E], F32, tag="xsum")
                nc.vector.tensor_reduce(
                    out=xsum[:],
                    in_=psel[:].rearrange("p (s w) -> p w s", s=NC_N),
                    axis=AX.X, op=OP.add)

                # ---------- xT ----------
                xT = sb.tile([128, KC * 64], F32, tag="xT")
                transpose_to(xT, xsum[:])

    nc.compile()
    return nc


_BUILT = {}


def _get_nc():
    if "nc" not in _BUILT:
        _BUILT["nc"] = build(T)
    return _BUILT["nc"]


def make_in_maps(z, emb, W_proj, b_proj, W_ih, b_ih, W_hh, b_hh, W_fc, b_fc):
    z = np.asarray(z, np.float32)
    emb = np.ascontiguousarray(np.asarray(emb, np.float32))
    W_proj = np.asarray(W_proj, np.float32)
    W_ih = np.asarray(W_ih, np.float32)
    W_hh = np.asarray(W_hh, np.float32)
    W_fc = np.asarray(W_fc, np.float32)
    b_proj = np.asarray(b_proj, np.float32)
    b_ih = np.asarray(b_ih, np.float32)
    b_hh = np.asarray(b_hh, np.float32)
    b_fc = np.asarray(b_fc, np.float32)

    wihT = np.ascontiguousarray(W_ih.T)            # [512, 1536]
    whhT = np.ascontiguousarray(W_hh.T)
    wprojT = np.ascontiguousarray(W_proj.T)        # [128, 512]
    zT = np.ascontiguousarray(z.T)                 # [128, 64]
    bias_gi = b_ih.copy()
    bias_gi[0:1024] += b_hh[0:1024]
    bias_gi = bias_gi[None, :]                     # [1, 1536]
    bias_hn = b_hh[None, 1024:1536]
    bias_proj = b_proj[None, :]

    common = dict(wprojT=wprojT, zT=zT,
                  bias_gi=bias_gi, bias_hn=bias_hn, bias_proj=bias_proj)
    in_maps = []
    for c in range(NC_N):
        wsh = np.ascontiguousarray(
            np.concatenate([wihT[64 * c:64 * c + 64, :],
                            whhT[64 * c:64 * c + 64, :]], axis=0))
        wfc_sh = W_fc[c * VS:(c + 1) * VS, :]          # [4000, 512]
        wfcT = np.zeros((H, VSP), np.float32)
        wfcT[:, 0:VS] = wfc_sh.T
        bias_fc = np.full((1, VSP), NEG, np.float32)
        bias_fc[0, 0:VS] = b_fc[c * VS:(c + 1) * VS]
        rank_col = np.full((B, 1), float(c * VS), np.float32)
        m = dict(common)
        m.update(wfcT=np.ascontiguousarray(wfcT), bias_fc=bias_fc,
                 rank_col=rank_col, wihwhh_sh=wsh,
                 embs=np.ascontiguousarray(emb[c * VS:(c + 1) * VS, :]))
        in_maps.append(m)
    return in_maps


def kernel(z, emb, W_proj, b_proj, W_ih, b_ih, W_hh, b_hh, W_fc, b_fc,
           context_length):
    assert int(context_length) == T
    nc = _get_nc()
    in_maps = make_in_maps(z, emb, W_proj, b_proj, W_ih, b_ih, W_hh, b_hh,
                           W_fc, b_fc)
    res = bass_utils.run_bass_kernel_spmd(nc, in_maps,
                                          core_ids=list(range(NC_N)))
    shards = [np.asarray(res.results[c]["out"]).astype(np.float32)
              .reshape(B, T, VS) for c in range(NC_N)]
    return np.concatenate(shards, axis=2)
